# revision 40
# baseline (speedup 1.0000x reference)
"""Trainium2 Bass kernel for grouped-query causal self-attention.

Problem shapes (hardcoded): x [8,1024,1024] f32, W_attn [6144,1024] f32,
W_proj [1024,4096] f32. 16 heads, head_dim 64, 4 query sets sharing one K/V.

Sharding: data parallel over batch — one batch element per NeuronCore (8 cores).
No collectives needed.

Per-core algorithm (everything "transposed" = [feature, token] layout so no
on-device transposes are needed; x is pre-transposed on the host):
  1. qkvT tiles = W_attn @ x^T   (stationary = W_attn^T tile, moving = x^T)
     -> K^T [1024f, 1024t], Q_g^T per set, and V in normal [t, f] layout.
  2. Attention per (set g, head h), computed transposed, 512-wide q chunks:
        S^T[k, q] = K_tile^T-stationary @ Q^T-moving   (contraction = head_dim)
        P^T = exp(S^T * scale)        (no max subtraction needed: logits ~N(0,1))
        causal: trim q-range per k-tile; zero the 128x128 diagonal triangle of
        P^T with a DVE multiply by a precomputed triangular 0/1 bf16 tile
        y_aug^T[d, q] = V_aug-stationary @ P^T-moving  (V augmented with a ones
        column -> row 64 of y_aug^T = softmax denominator, for free)
        normalize: denominator row -> partition-0 SBUF copy, DVE
        reciprocal_approx_fast, gpsimd partition_broadcast to 64 lanes, DVE
        multiply straight from PSUM -> yt (bf16). The multiply is deferred so
        the DVE queue never blocks on the gpsimd broadcast.
  3. out = combined @ W_proj^T accumulated over sets (stationary = y^T tiles,
     moving = W_proj^T streamed from DRAM), bf16 SBUF accumulator.

PE array tiling: head_dim=64 so each S matmul only uses 64 contraction rows.
Heads are processed in (even, odd) pairs whose K^T/Q^T live in partitions
0-63 / 64-127 of the same tiles: bass auto-derives tile_position (0,0)/(64,0)
with tile_size (64,128), and adjacent-emitted pair matmuls run CONCURRENTLY
in the two 64x128 row-tiles of the PE array (~2x on the S stretch).

Engine schedule: the Scalar (ACT) engine runs *only* Exp (table reloads cost
1.3us). The PE is kept continuously busy (p-state: any idle gap drops it to
1.2GHz for the next 3us) by interleaving filler projection groups (Q-proj of
set g+1, out-proj of set g-1, and the moved second half of the set-0 Q-proj)
into each attention pair-slot, paced so the ACT exp queue is always primed.
qt/yt are double-buffered across sets.

dtypes: bf16 operands for matmuls (fp32 PSUM accumulate), fp32 softmax
denominator path, bf16 output accumulation (headroom: rel err ~9e-3 vs 2e-2).
"""

import math

import ml_dtypes
import numpy as np

import concourse.bacc as bacc
import concourse.bass as bass
import concourse.mybir as mybir
import concourse.tile as tile
from concourse.bass_utils import run_bass_kernel_spmd

BF16 = ml_dtypes.bfloat16

B, T, C = 8, 1024, 1024
NH, HD, NQS = 16, 64, 4
SCALE = 1.0 / math.sqrt(HD)
NT = T // 128  # token tiles
NCH = C // 128  # channel tiles
KOFF = NQS * C  # 4096: K rows in W_attn
VOFF = (NQS + 1) * C  # 5120: V rows in W_attn

_CACHE = {}
LAST = {}  # exec_time_ns etc for test harness


def _build():
    f32 = mybir.dt.float32
    bf16 = mybir.dt.bfloat16
    EXP = mybir.ActivationFunctionType.Exp

    nc = bacc.Bacc()
    xT = nc.declare_dram_parameter("xT", [C, T], bf16, isOutput=False)
    waT = nc.declare_dram_parameter("waT", [C, 6 * C], bf16, isOutput=False)
    wpT = nc.declare_dram_parameter("wpT", [NQS * C, C], bf16, isOutput=False)
    triD = nc.declare_dram_parameter("tri", [128, 128], bf16, isOutput=False)
    out = nc.declare_dram_parameter("out", [T, C], bf16, isOutput=True)

    with tile.TileContext(nc) as tc:
        with (
            tc.tile_pool(name="res", bufs=1) as res,
            tc.tile_pool(name="wa", bufs=24) as wa_pool,
            tc.tile_pool(name="wp", bufs=16) as wp_pool,
            tc.tile_pool(name="pt", bufs=24) as pt_pool,
            tc.tile_pool(name="small", bufs=2) as small_pool,
            tc.tile_pool(name="dsm", bufs=2) as den_pool,
            tc.tile_pool(name="psp", bufs=6, space="PSUM") as psp,  # S + filler tiles
            tc.tile_pool(name="pyp", bufs=2, space="PSUM") as pyp,  # AV accum
        ):
            xt = [res.tile([128, T], bf16, tag=f"xt{i}", name=f"xt{i}") for i in range(NCH)]
            kt = [res.tile([128, T], bf16, tag=f"kt{i}", name=f"kt{i}") for i in range(NCH)]
            vt = [res.tile([128, NH, HD + 1], bf16, tag=f"vt{i}", name=f"vt{i}") for i in range(NT)]
            qt = [
                [res.tile([128, T], bf16, tag=f"qt{b}_{i}", name=f"qt{b}_{i}") for i in range(NCH)]
                for b in range(2)
            ]
            yt = [
                [res.tile([128, T], bf16, tag=f"yt{b}_{i}", name=f"yt{b}_{i}") for i in range(NCH)]
                for b in range(2)
            ]
            osb = [res.tile([128, C], bf16, tag=f"osb{i}", name=f"osb{i}") for i in range(NT)]

            # xt first: the first Kproj matmuls wait on these; the ones
            # column and mask tile are cheap and not needed until attention.
            for i in range(NCH):
                nc.sync.dma_start(out=xt[i], in_=xT[i * 128 : (i + 1) * 128, :])
            tri = res.tile([128, 128], bf16, tag="tri", name="tri")
            nc.sync.dma_start(out=tri, in_=triD[:, :])
            for tt in range(NT):
                nc.gpsimd.memset(vt[tt][:, :, HD : HD + 1], 1.0)

            def load_wa(fbase, fg, tag):
                was = []
                for ct in range(NCH):
                    w = wa_pool.tile([128, 512], bf16, tag="wa", name=f"wa_{tag}_{fg}_{ct}")
                    f0 = fbase + fg * 512
                    nc.sync.dma_start(
                        out=w, in_=waT[ct * 128 : (ct + 1) * 128, f0 : f0 + 512]
                    )
                    was.append(w)
                return was

            def proj_group(dst, was, fg, grp, tag, pool):
                """One PSUM group of a [feature, token] projection: 8 matmuls
                + 1 cast copy. grp in 0..7 -> (tc2, ftl)."""
                tc2, ftl = grp // 4, grp % 4
                ps = pool.tile([128, 512], f32, tag=pool.name, name=f"ps_{tag}_{fg}_{grp}")
                for ct in range(NCH):
                    nc.tensor.matmul(
                        ps,
                        was[ct][:, ftl * 128 : (ftl + 1) * 128],
                        xt[ct][:, tc2 * 512 : (tc2 + 1) * 512],
                        start=(ct == 0),
                        stop=(ct == NCH - 1),
                    )
                fti = fg * 4 + ftl
                nc.vector.tensor_copy(dst[fti][:, tc2 * 512 : (tc2 + 1) * 512], ps)

            def load_wp(g, cc):
                wps = []
                for ftl in range(NCH):
                    w = wp_pool.tile([128, 512], bf16, tag="wp", name=f"wp{g}_{cc}_{ftl}")
                    nc.sync.dma_start(
                        out=w,
                        in_=wpT[
                            g * C + ftl * 128 : g * C + (ftl + 1) * 128,
                            cc * 512 : (cc + 1) * 512,
                        ],
                    )
                    wps.append(w)
                return wps

            def outproj_group(ytb, wps, g, cc, tt, pool):
                ps = pool.tile([128, 512], f32, tag=pool.name, name=f"psp{g}_{cc}_{tt}")
                for ftl in range(NCH):
                    nc.tensor.matmul(
                        ps,
                        ytb[ftl][:, tt * 128 : (tt + 1) * 128],
                        wps[ftl],
                        start=(ftl == 0),
                        stop=(ftl == NCH - 1),
                    )
                dst = osb[tt][:, cc * 512 : (cc + 1) * 512]
                if g == 0:
                    nc.vector.tensor_copy(dst, ps)
                else:
                    nc.vector.tensor_add(dst, dst, ps)

            # ---- prologue: K, V, Q0-fg0 projections (PE-dense, no exp) ----
            for fg in range(2):
                was = load_wa(KOFF, fg, "k")
                for grp in range(8):
                    proj_group(kt, was, fg, grp, "k", psp)
            for fg in range(2):
                was = load_wa(VOFF, fg, "v")
                for tt in range(NT):
                    ps = psp.tile([128, 512], f32, tag="psp", name=f"psv_{fg}_{tt}")
                    for ct in range(NCH):
                        nc.tensor.matmul(
                            ps,
                            xt[ct][:, tt * 128 : (tt + 1) * 128],
                            was[ct],
                            start=(ct == 0),
                            stop=(ct == NCH - 1),
                        )
                    nc.vector.tensor_copy(
                        vt[tt][:, fg * 8 : (fg + 1) * 8, 0:HD],
                        ps.rearrange("p (a b) -> p a b", b=HD),
                    )
            was_q0f0 = load_wa(0, 0, "q0")
            for grp in range(8):
                proj_group(qt[0], was_q0f0, 0, grp, "q0", psp)
            # Q0-fg1 W tiles: their groups run as set-0 slot fillers
            wa_tiles = {(0, 1): load_wa(0, 1, "q0b"), (1, 0): load_wa(C, 0, "q1")}
            wp_tiles = {}

            # ---- main loop: head-pair slots ----
            def s_block(g, h, qc, k2, pts):
                """S^T matmul + exp (+ diag mask) for one 128-k tile."""
                ft, ro = h // 2, (h % 2) * 64
                qlo = max(qc * 512, k2 * 128)
                w = qc * 512 + 512 - qlo
                sp = psp.tile([128, 512], f32, tag="psp", name=f"sp{g}_{h}_{qc}_{k2}")
                nc.tensor.matmul(
                    sp[:, :w],
                    kt[ft][ro : ro + 64, k2 * 128 : (k2 + 1) * 128],
                    qt[g % 2][ft][ro : ro + 64, qlo : qlo + w],
                    start=True,
                    stop=True,
                )
                pt = pt_pool.tile([128, 512], bf16, tag="pt", name=f"pt{g}_{h}_{qc}_{k2}")
                nc.scalar.activation(pt[:, :w], sp[:, :w], EXP, bias=0.0, scale=SCALE)
                if k2 * 128 >= qc * 512:  # diagonal block: zero upper triangle
                    nc.vector.tensor_mul(pt[:, 0:128], pt[:, 0:128], tri)
                pts[(h, qc, k2)] = pt

            def s_pair(g, hp, qc, k2a, k2b, pts):
                """Adjacent-emitted S blocks for the (even, odd) head pair ->
                the two 64x128 PE row-tiles run them concurrently."""
                for k2 in (k2a, k2b):
                    if k2 is None:
                        continue
                    s_block(g, 2 * hp, qc, k2, pts)
                    s_block(g, 2 * hp + 1, qc, k2, pts)

            def av_chain(g, h, qc, k2s, nkt, yp, pts):
                for k2 in k2s:
                    qlo = max(qc * 512, k2 * 128)
                    w = qc * 512 + 512 - qlo
                    off = qlo - qc * 512
                    nc.tensor.matmul(
                        yp[0:65, off : off + w],
                        vt[k2][:, h, :],
                        pts[(h, qc, k2)][:, :w],
                        start=(k2 == 0),
                        stop=(k2 == nkt - 1),
                    )

            def norm_start(g, h, qc, yp):
                """Denominator -> reciprocal -> broadcast; returns the deferred
                final multiply (emitted later so the DVE queue doesn't block
                on the gpsimd broadcast)."""
                ft, ro = h // 2, (h % 2) * 64
                den = den_pool.tile([1, 512], f32, tag="den", name=f"den{g}_{h}_{qc}")
                nc.vector.tensor_copy(den, yp[64:65, :])
                rec = small_pool.tile([1, 512], f32, tag="rec", name=f"rec{g}_{h}_{qc}")
                nc.vector.reciprocal_approx_fast(out=rec, in_=den)
                bcst = small_pool.tile([64, 512], f32, tag="bcst", name=f"bcst{g}_{h}_{qc}")
                nc.gpsimd.partition_broadcast(bcst, rec)

                def mul():
                    nc.vector.tensor_mul(
                        yt[g % 2][ft][ro : ro + 64, qc * 512 : qc * 512 + 512],
                        yp[0:64, :],
                        bcst,
                    )

                return mul

            carry = []  # normalize muls deferred across the slot boundary
            for g in range(NQS):
                for hp in range(8):  # head pair: heads 2hp, 2hp+1
                    h0, h1 = 2 * hp, 2 * hp + 1
                    # --- staggered W DMAs, ~2 pair-slots of lead ---
                    if g < 3 and hp == 2:  # qproj(g+1) fg1, fillers F0/F1 slots 4-7
                        wa_tiles[(g + 1, 1)] = load_wa((g + 1) * C, 1, f"q{g + 1}")
                    if g < 2 and hp == 6:  # qproj(g+2) fg0, next set slots 0-3
                        wa_tiles[(g + 2, 0)] = load_wa((g + 2) * C, 0, f"q{g + 2}")
                    if g > 0 and hp == 2:  # outproj(g-1) cc1, F2/F3 slots 4-7
                        wp_tiles[(g - 1, 1)] = load_wp(g - 1, 1)
                    if g < 3 and hp == 6:  # outproj(g) cc0, next set slots 0-3
                        wp_tiles[(g, 0)] = load_wp(g, 0)
                    if g == 3 and hp == 6:  # epilogue outproj(3) cc0
                        wp_tiles[(3, 0)] = load_wp(3, 0)

                    # Filler groups for this pair-slot, emitted as HALF-chains
                    # (4 matmuls each, both halves of one PSUM accumulation
                    # group) at up to 8 pacing positions through the slot.
                    halves = []

                    def add_proj(dst, was, fg, grp, tag):
                        tc2, ftl = grp // 4, grp % 4
                        box = []

                        def h1():
                            ps = psp.tile([128, 512], f32, tag="psp",
                                          name=f"ps_{tag}_{fg}_{grp}")
                            box.append(ps)
                            for ct in range(4):
                                nc.tensor.matmul(
                                    ps, was[ct][:, ftl * 128 : (ftl + 1) * 128],
                                    xt[ct][:, tc2 * 512 : (tc2 + 1) * 512],
                                    start=(ct == 0), stop=False,
                                )

                        def h2():
                            ps = box[0]
                            for ct in range(4, NCH):
                                nc.tensor.matmul(
                                    ps, was[ct][:, ftl * 128 : (ftl + 1) * 128],
                                    xt[ct][:, tc2 * 512 : (tc2 + 1) * 512],
                                    start=False, stop=(ct == NCH - 1),
                                )
                            fti = fg * 4 + ftl
                            nc.vector.tensor_copy(
                                dst[fti][:, tc2 * 512 : (tc2 + 1) * 512], ps
                            )

                        halves.extend([h1, h2])

                    def add_outproj(ytb, wps, go, cc, tt):
                        box = []

                        def h1():
                            ps = psp.tile([128, 512], f32, tag="psp",
                                          name=f"psp{go}_{cc}_{tt}")
                            box.append(ps)
                            for ftl in range(4):
                                nc.tensor.matmul(
                                    ps, ytb[ftl][:, tt * 128 : (tt + 1) * 128],
                                    wps[ftl], start=(ftl == 0), stop=False,
                                )

                        def h2():
                            ps = box[0]
                            for ftl in range(4, NCH):
                                nc.tensor.matmul(
                                    ps, ytb[ftl][:, tt * 128 : (tt + 1) * 128],
                                    wps[ftl], start=False, stop=(ftl == NCH - 1),
                                )
                            dst = osb[tt][:, cc * 512 : (cc + 1) * 512]
                            if go == 0:
                                nc.vector.tensor_copy(dst, ps)
                            else:
                                nc.vector.tensor_add(dst, dst, ps)

                        halves.extend([h1, h2])

                    if g < 3:  # Qproj(g+1) groups 2hp, 2hp+1
                        fg = hp // 4
                        for grp in (2 * hp - 8 * fg, 2 * hp + 1 - 8 * fg):
                            add_proj(qt[(g + 1) % 2], wa_tiles[(g + 1, fg)], fg,
                                     grp, f"q{g + 1}")
                    if g == 0 and hp < 4:  # moved Q0-fg1 groups
                        for grp in (2 * hp, 2 * hp + 1):
                            add_proj(qt[0], wa_tiles[(0, 1)], 1, grp, "q0b")
                    if g > 0:  # outproj(g-1) groups 2hp, 2hp+1
                        cc = hp // 4
                        for tt in (2 * hp - 8 * cc, 2 * hp + 1 - 8 * cc):
                            add_outproj(yt[(g - 1) % 2], wp_tiles[(g - 1, cc)],
                                        g - 1, cc, tt)
                    # whole filler groups (both halves back-to-back) at the
                    # proven positions: 4-group slots at p0/p2/p6/p7, 2-group
                    # slots at p2/p6 (late enough to dodge the slot-boundary
                    # ppj WAR, and inside the AV qc1 chains)
                    sched = {0, 2, 6, 7} if len(halves) == 8 else {0, 6}
                    pos = [0]

                    def filler():
                        if pos[0] in sched and halves:
                            halves.pop(0)()
                            halves.pop(0)()
                        pos[0] += 1

                    pts = {}
                    # S streams, pair-interleaved; filler halves pace the ACT
                    # queue; tail normalize-muls carried from the previous slot
                    s_pair(g, hp, 0, 0, 1, pts)
                    filler()  # p0
                    for m in carry:
                        m()
                    carry = []
                    s_pair(g, hp, 0, 2, 3, pts)
                    filler()  # p1
                    s_pair(g, hp, 1, 0, 1, pts)
                    filler()  # p2
                    s_pair(g, hp, 1, 2, 3, pts)
                    filler()  # p3
                    # AV qc0 head h0
                    yp00 = pyp.tile([128, 512], f32, tag="pyp", name=f"yp{g}_{h0}_0")
                    av_chain(g, h0, 0, range(4), 4, yp00, pts)
                    mul00 = norm_start(g, h0, 0, yp00)
                    s_pair(g, hp, 1, 4, 5, pts)
                    filler()  # p4
                    # AV qc0 head h1
                    yp10 = pyp.tile([128, 512], f32, tag="pyp", name=f"yp{g}_{h1}_0")
                    av_chain(g, h1, 0, range(4), 4, yp10, pts)
                    mul10 = norm_start(g, h1, 0, yp10)
                    s_pair(g, hp, 1, 6, 7, pts)
                    filler()  # p5
                    mul00()  # its broadcast is done; frees yp00 for yp11
                    # AV qc1 head h0, filler half inside the accumulation chain
                    yp01 = pyp.tile([128, 512], f32, tag="pyp", name=f"yp{g}_{h0}_1")
                    av_chain(g, h0, 1, range(4), 8, yp01, pts)
                    filler()  # p6
                    av_chain(g, h0, 1, range(4, 8), 8, yp01, pts)
                    mul01 = norm_start(g, h0, 1, yp01)
                    mul10()
                    # AV qc1 head h1
                    yp11 = pyp.tile([128, 512], f32, tag="pyp", name=f"yp{g}_{h1}_1")
                    av_chain(g, h1, 1, range(4), 8, yp11, pts)
                    filler()  # p7
                    av_chain(g, h1, 1, range(4, 8), 8, yp11, pts)
                    mul11 = norm_start(g, h1, 1, yp11)
                    carry = [mul01, mul11]

            # ---- epilogue: outproj(set 3) + store ----
            for m in carry:
                m()
            wp_tiles[(3, 1)] = load_wp(3, 1)  # hides under the cc0 groups
            for cc in range(2):
                for tt in range(NT):
                    outproj_group(yt[3 % 2], wp_tiles[(3, cc)], 3, cc, tt, psp)
                    if cc == 1:  # osb[tt] final -> overlap store with compute
                        nc.sync.dma_start(
                            out=out[tt * 128 : (tt + 1) * 128, :], in_=osb[tt]
                        )

    nc.compile()
    return nc


def kernel(x, W_attn, W_proj, _trace=False):
    if "nc" not in _CACHE:
        _CACHE["nc"] = _build()
    nc = _CACHE["nc"]

    xT = np.ascontiguousarray(np.transpose(np.asarray(x, np.float32), (0, 2, 1))).astype(BF16)
    waT = np.ascontiguousarray(np.asarray(W_attn, np.float32).T).astype(BF16)
    wpT = np.ascontiguousarray(np.asarray(W_proj, np.float32).T).astype(BF16)
    ii = np.arange(128)
    # P^T[k, q] keep (multiply by 1) where q >= k
    tri = (ii[None, :] >= ii[:, None]).astype(np.float32).astype(BF16)

    in_maps = [
        {"xT": xT[b], "waT": waT, "wpT": wpT, "tri": tri}
        for b in range(B)
    ]
    res = run_bass_kernel_spmd(nc, in_maps, core_ids=list(range(B)), trace=_trace)
    LAST["exec_time_ns"] = res.exec_time_ns
    LAST["mean_exec_time_ns"] = res.mean_exec_time_ns
    LAST["results"] = res
    return np.stack([res.results[b]["out"] for b in range(B)]).astype(np.float32)


# revision 41
# speedup vs baseline: 1.1103x; 1.1103x over previous
"""Trainium2 Bass kernel for grouped-query causal self-attention.

Problem shapes (hardcoded): x [8,1024,1024] f32, W_attn [6144,1024] f32,
W_proj [1024,4096] f32. 16 heads, head_dim 64, 4 query sets sharing one K/V.

Sharding: data parallel over batch — one batch element per NeuronCore (8 cores).
No collectives needed.

Per-core algorithm (everything "transposed" = [feature, token] layout so no
on-device transposes are needed; x is pre-transposed on the host):
  1. qkvT tiles = W_attn @ x^T   (stationary = W_attn^T tile, moving = x^T)
     -> K^T [1024f, 1024t], Q_g^T per set, and V in normal [t, f] layout.
  2. Attention per (set g, head h), computed transposed, 512-wide q chunks:
        S^T[k, q] = K_tile^T-stationary @ Q^T-moving   (contraction = head_dim)
        P^T = exp(S^T * scale)        (no max subtraction needed: logits ~N(0,1))
        causal: trim q-range per k-tile; zero the 128x128 diagonal triangle of
        P^T with a DVE multiply by a precomputed triangular 0/1 bf16 tile
        y_aug^T[d, q] = V_aug-stationary @ P^T-moving  (V augmented with a ones
        column -> row 64 of y_aug^T = softmax denominator, for free)
        normalize: denominator row -> partition-0 SBUF copy, DVE
        reciprocal_approx_fast, gpsimd partition_broadcast to 64 lanes, DVE
        multiply straight from PSUM -> yt (bf16). The multiply is deferred so
        the DVE queue never blocks on the gpsimd broadcast.
  3. out = combined @ W_proj^T accumulated over sets (stationary = y^T tiles,
     moving = W_proj^T streamed from DRAM), bf16 SBUF accumulator.

PE array tiling: head_dim=64 so each S matmul only uses 64 contraction rows.
Heads are processed in (even, odd) pairs whose K^T/Q^T live in partitions
0-63 / 64-127 of the same tiles: bass auto-derives tile_position (0,0)/(64,0)
with tile_size (64,128), and adjacent-emitted pair matmuls run CONCURRENTLY
in the two 64x128 row-tiles of the PE array (~2x on the S stretch).

Engine schedule: the Scalar (ACT) engine runs *only* Exp (table reloads cost
1.3us). The PE is kept continuously busy (p-state: any idle gap drops it to
1.2GHz for the next 3us) by interleaving filler projection groups (Q-proj of
set g+1, out-proj of set g-1, and the moved second half of the set-0 Q-proj)
into each attention pair-slot, paced so the ACT exp queue is always primed.
qt/yt are double-buffered across sets.

dtypes: bf16 operands for matmuls (fp32 PSUM accumulate), fp32 softmax
denominator path, bf16 output accumulation (headroom: rel err ~9e-3 vs 2e-2).
"""

import math

import ml_dtypes
import numpy as np

import concourse.bacc as bacc
import concourse.bass as bass
import concourse.mybir as mybir
import concourse.tile as tile
from concourse.bass_utils import run_bass_kernel_spmd

BF16 = ml_dtypes.bfloat16

B, T, C = 8, 1024, 1024
NH, HD, NQS = 16, 64, 4
SCALE = 1.0 / math.sqrt(HD)
NT = T // 128  # token tiles
NCH = C // 128  # channel tiles
KOFF = NQS * C  # 4096: K rows in W_attn
VOFF = (NQS + 1) * C  # 5120: V rows in W_attn

_CACHE = {}
LAST = {}  # exec_time_ns etc for test harness


def _build():
    f32 = mybir.dt.float32
    bf16 = mybir.dt.bfloat16
    EXP = mybir.ActivationFunctionType.Exp

    nc = bacc.Bacc()
    xT = nc.declare_dram_parameter("xT", [C, T], bf16, isOutput=False)
    waT = nc.declare_dram_parameter("waT", [C, 6 * C], bf16, isOutput=False)
    wpT = nc.declare_dram_parameter("wpT", [NQS * C, C], bf16, isOutput=False)
    triD = nc.declare_dram_parameter("tri", [128, 128], bf16, isOutput=False)
    out = nc.declare_dram_parameter("out", [T, C], bf16, isOutput=True)

    with tile.TileContext(nc) as tc:
        with (
            tc.tile_pool(name="res", bufs=1) as res,
            tc.tile_pool(name="wa", bufs=24) as wa_pool,
            tc.tile_pool(name="wp", bufs=16) as wp_pool,
            tc.tile_pool(name="pt", bufs=24) as pt_pool,
            tc.tile_pool(name="small", bufs=2) as small_pool,
            tc.tile_pool(name="dsm", bufs=2) as den_pool,
            tc.tile_pool(name="psp", bufs=5, space="PSUM") as psp,  # S tiles
            tc.tile_pool(name="pyp", bufs=2, space="PSUM") as pyp,  # AV accum
            tc.tile_pool(name="ppj", bufs=1, space="PSUM") as ppj,  # projections
        ):
            xt = [res.tile([128, T], bf16, tag=f"xt{i}", name=f"xt{i}") for i in range(NCH)]
            kt = [res.tile([128, T], bf16, tag=f"kt{i}", name=f"kt{i}") for i in range(NCH)]
            vt = [res.tile([128, NH, HD + 1], bf16, tag=f"vt{i}", name=f"vt{i}") for i in range(NT)]
            qt = [
                [res.tile([128, T], bf16, tag=f"qt{b}_{i}", name=f"qt{b}_{i}") for i in range(NCH)]
                for b in range(2)
            ]
            yt = [
                [res.tile([128, T], bf16, tag=f"yt{b}_{i}", name=f"yt{b}_{i}") for i in range(NCH)]
                for b in range(2)
            ]
            osb = [res.tile([128, C], bf16, tag=f"osb{i}", name=f"osb{i}") for i in range(NT)]

            # xt first: the first Kproj matmuls wait on these; the ones
            # column and mask tile are cheap and not needed until attention.
            for i in range(NCH):
                nc.sync.dma_start(out=xt[i], in_=xT[i * 128 : (i + 1) * 128, :])
            tri = res.tile([128, 128], bf16, tag="tri", name="tri")
            nc.sync.dma_start(out=tri, in_=triD[:, :])
            for tt in range(NT):
                nc.gpsimd.memset(vt[tt][:, :, HD : HD + 1], 1.0)

            def load_wa(fbase, fg, tag):
                was = []
                for ct in range(NCH):
                    w = wa_pool.tile([128, 512], bf16, tag="wa", name=f"wa_{tag}_{fg}_{ct}")
                    f0 = fbase + fg * 512
                    nc.sync.dma_start(
                        out=w, in_=waT[ct * 128 : (ct + 1) * 128, f0 : f0 + 512]
                    )
                    was.append(w)
                return was

            def proj_group(dst, was, fg, grp, tag, pool):
                """One PSUM group of a [feature, token] projection: 8 matmuls
                + 1 cast copy. grp in 0..7 -> (tc2, ftl)."""
                tc2, ftl = grp // 4, grp % 4
                ps = pool.tile([128, 512], f32, tag=pool.name, name=f"ps_{tag}_{fg}_{grp}")
                for ct in range(NCH):
                    nc.tensor.matmul(
                        ps,
                        was[ct][:, ftl * 128 : (ftl + 1) * 128],
                        xt[ct][:, tc2 * 512 : (tc2 + 1) * 512],
                        start=(ct == 0),
                        stop=(ct == NCH - 1),
                    )
                fti = fg * 4 + ftl
                nc.vector.tensor_copy(dst[fti][:, tc2 * 512 : (tc2 + 1) * 512], ps)

            def load_wp(g, cc):
                wps = []
                for ftl in range(NCH):
                    w = wp_pool.tile([128, 512], bf16, tag="wp", name=f"wp{g}_{cc}_{ftl}")
                    nc.sync.dma_start(
                        out=w,
                        in_=wpT[
                            g * C + ftl * 128 : g * C + (ftl + 1) * 128,
                            cc * 512 : (cc + 1) * 512,
                        ],
                    )
                    wps.append(w)
                return wps

            def outproj_group(ytb, wps, g, cc, tt, pool):
                ps = pool.tile([128, 512], f32, tag=pool.name, name=f"psp{g}_{cc}_{tt}")
                for ftl in range(NCH):
                    nc.tensor.matmul(
                        ps,
                        ytb[ftl][:, tt * 128 : (tt + 1) * 128],
                        wps[ftl],
                        start=(ftl == 0),
                        stop=(ftl == NCH - 1),
                    )
                dst = osb[tt][:, cc * 512 : (cc + 1) * 512]
                if g == 0:
                    nc.vector.tensor_copy(dst, ps)
                else:
                    nc.vector.tensor_add(dst, dst, ps)

            # ---- prologue: K, V, Q0-fg0 projections (PE-dense, no exp) ----
            for fg in range(2):
                was = load_wa(KOFF, fg, "k")
                for grp in range(8):
                    proj_group(kt, was, fg, grp, "k", psp)
            for fg in range(2):
                was = load_wa(VOFF, fg, "v")
                for tt in range(NT):
                    ps = psp.tile([128, 512], f32, tag="psp", name=f"psv_{fg}_{tt}")
                    for ct in range(NCH):
                        nc.tensor.matmul(
                            ps,
                            xt[ct][:, tt * 128 : (tt + 1) * 128],
                            was[ct],
                            start=(ct == 0),
                            stop=(ct == NCH - 1),
                        )
                    nc.vector.tensor_copy(
                        vt[tt][:, fg * 8 : (fg + 1) * 8, 0:HD],
                        ps.rearrange("p (a b) -> p a b", b=HD),
                    )
            was_q0f0 = load_wa(0, 0, "q0")
            for grp in range(8):
                proj_group(qt[0], was_q0f0, 0, grp, "q0", psp)
            # Q0-fg1 W tiles: their groups run as set-0 slot fillers
            wa_tiles = {(0, 1): load_wa(0, 1, "q0b"), (1, 0): load_wa(C, 0, "q1")}
            wp_tiles = {}

            # ---- main loop: head-pair slots ----
            def s_block(g, h, qc, k2, pts):
                """S^T matmul + exp (+ diag mask) for one 128-k tile."""
                ft, ro = h // 2, (h % 2) * 64
                qlo = max(qc * 512, k2 * 128)
                w = qc * 512 + 512 - qlo
                sp = psp.tile([128, 512], f32, tag="psp", name=f"sp{g}_{h}_{qc}_{k2}")
                nc.tensor.matmul(
                    sp[:, :w],
                    kt[ft][ro : ro + 64, k2 * 128 : (k2 + 1) * 128],
                    qt[g % 2][ft][ro : ro + 64, qlo : qlo + w],
                    start=True,
                    stop=True,
                )
                pt = pt_pool.tile([128, 512], bf16, tag="pt", name=f"pt{g}_{h}_{qc}_{k2}")
                nc.scalar.activation(pt[:, :w], sp[:, :w], EXP, bias=0.0, scale=SCALE)
                if k2 * 128 >= qc * 512:  # diagonal block: zero upper triangle
                    nc.vector.tensor_mul(pt[:, 0:128], pt[:, 0:128], tri)
                pts[(h, qc, k2)] = pt

            def s_pair(g, hp, qc, k2a, k2b, pts):
                """Adjacent-emitted S blocks for the (even, odd) head pair ->
                the two 64x128 PE row-tiles run them concurrently."""
                for k2 in (k2a, k2b):
                    if k2 is None:
                        continue
                    s_block(g, 2 * hp, qc, k2, pts)
                    s_block(g, 2 * hp + 1, qc, k2, pts)

            def av_chain(g, h, qc, k2s, nkt, yp, pts):
                for k2 in k2s:
                    qlo = max(qc * 512, k2 * 128)
                    w = qc * 512 + 512 - qlo
                    off = qlo - qc * 512
                    nc.tensor.matmul(
                        yp[0:65, off : off + w],
                        vt[k2][:, h, :],
                        pts[(h, qc, k2)][:, :w],
                        start=(k2 == 0),
                        stop=(k2 == nkt - 1),
                    )

            def norm_start(g, h, qc, yp):
                """Denominator -> reciprocal -> broadcast; returns the deferred
                final multiply (emitted later so the DVE queue doesn't block
                on the gpsimd broadcast)."""
                ft, ro = h // 2, (h % 2) * 64
                den = den_pool.tile([1, 512], f32, tag="den", name=f"den{g}_{h}_{qc}")
                nc.vector.tensor_copy(den, yp[64:65, :])
                rec = small_pool.tile([1, 512], f32, tag="rec", name=f"rec{g}_{h}_{qc}")
                nc.vector.reciprocal_approx_fast(out=rec, in_=den)
                bcst = small_pool.tile([64, 512], f32, tag="bcst", name=f"bcst{g}_{h}_{qc}")
                nc.gpsimd.partition_broadcast(bcst, rec)

                def mul():
                    nc.vector.tensor_mul(
                        yt[g % 2][ft][ro : ro + 64, qc * 512 : qc * 512 + 512],
                        yp[0:64, :],
                        bcst,
                    )

                return mul

            carry = []  # normalize muls deferred across the slot boundary
            for g in range(NQS):
                for hp in range(8):  # head pair: heads 2hp, 2hp+1
                    h0, h1 = 2 * hp, 2 * hp + 1
                    # --- staggered W DMAs, ~2 pair-slots of lead ---
                    if g < 3 and hp == 2:  # qproj(g+1) fg1, fillers F0/F1 slots 4-7
                        wa_tiles[(g + 1, 1)] = load_wa((g + 1) * C, 1, f"q{g + 1}")
                    if g < 2 and hp == 6:  # qproj(g+2) fg0, next set slots 0-3
                        wa_tiles[(g + 2, 0)] = load_wa((g + 2) * C, 0, f"q{g + 2}")
                    if g > 0 and hp == 2:  # outproj(g-1) cc1, F2/F3 slots 4-7
                        wp_tiles[(g - 1, 1)] = load_wp(g - 1, 1)
                    if g < 3 and hp == 6:  # outproj(g) cc0, next set slots 0-3
                        wp_tiles[(g, 0)] = load_wp(g, 0)
                    if g == 3 and hp == 6:  # epilogue outproj(3) cc0
                        wp_tiles[(3, 0)] = load_wp(3, 0)

                    # Filler groups for this pair-slot, emitted as HALF-chains
                    # (4 matmuls each, both halves of one PSUM accumulation
                    # group) at up to 8 pacing positions through the slot.
                    halves = []

                    def add_proj(dst, was, fg, grp, tag):
                        tc2, ftl = grp // 4, grp % 4
                        box = []

                        def h1():
                            ps = ppj.tile([128, 512], f32, tag="ppj",
                                          name=f"ps_{tag}_{fg}_{grp}")
                            box.append(ps)
                            for ct in range(4):
                                nc.tensor.matmul(
                                    ps, was[ct][:, ftl * 128 : (ftl + 1) * 128],
                                    xt[ct][:, tc2 * 512 : (tc2 + 1) * 512],
                                    start=(ct == 0), stop=False,
                                )

                        def h2():
                            ps = box[0]
                            for ct in range(4, NCH):
                                nc.tensor.matmul(
                                    ps, was[ct][:, ftl * 128 : (ftl + 1) * 128],
                                    xt[ct][:, tc2 * 512 : (tc2 + 1) * 512],
                                    start=False, stop=(ct == NCH - 1),
                                )
                            fti = fg * 4 + ftl
                            nc.vector.tensor_copy(
                                dst[fti][:, tc2 * 512 : (tc2 + 1) * 512], ps
                            )

                        halves.extend([h1, h2])

                    def add_outproj(ytb, wps, go, cc, tt):
                        box = []

                        def h1():
                            ps = ppj.tile([128, 512], f32, tag="ppj",
                                          name=f"psp{go}_{cc}_{tt}")
                            box.append(ps)
                            for ftl in range(4):
                                nc.tensor.matmul(
                                    ps, ytb[ftl][:, tt * 128 : (tt + 1) * 128],
                                    wps[ftl], start=(ftl == 0), stop=False,
                                )

                        def h2():
                            ps = box[0]
                            for ftl in range(4, NCH):
                                nc.tensor.matmul(
                                    ps, ytb[ftl][:, tt * 128 : (tt + 1) * 128],
                                    wps[ftl], start=False, stop=(ftl == NCH - 1),
                                )
                            dst = osb[tt][:, cc * 512 : (cc + 1) * 512]
                            if go == 0:
                                nc.vector.tensor_copy(dst, ps)
                            else:
                                nc.vector.tensor_add(dst, dst, ps)

                        halves.extend([h1, h2])

                    if g < 3:  # Qproj(g+1) groups 2hp, 2hp+1
                        fg = hp // 4
                        for grp in (2 * hp - 8 * fg, 2 * hp + 1 - 8 * fg):
                            add_proj(qt[(g + 1) % 2], wa_tiles[(g + 1, fg)], fg,
                                     grp, f"q{g + 1}")
                    if g == 0 and hp < 4:  # moved Q0-fg1 groups
                        for grp in (2 * hp, 2 * hp + 1):
                            add_proj(qt[0], wa_tiles[(0, 1)], 1, grp, "q0b")
                    if g > 0:  # outproj(g-1) groups 2hp, 2hp+1
                        cc = hp // 4
                        for tt in (2 * hp - 8 * cc, 2 * hp + 1 - 8 * cc):
                            add_outproj(yt[(g - 1) % 2], wp_tiles[(g - 1, cc)],
                                        g - 1, cc, tt)
                    # whole filler groups (both halves back-to-back) at the
                    # proven positions: 4-group slots at p0/p2/p6/p7, 2-group
                    # slots at p2/p6 (late enough to dodge the slot-boundary
                    # ppj WAR, and inside the AV qc1 chains)
                    sched = {0, 2, 6, 7} if len(halves) == 8 else {0, 6}
                    pos = [0]

                    def filler():
                        if pos[0] in sched and halves:
                            halves.pop(0)()
                            halves.pop(0)()
                        pos[0] += 1

                    pts = {}
                    # S streams, pair-interleaved; filler halves pace the ACT
                    # queue; tail normalize-muls carried from the previous slot
                    s_pair(g, hp, 0, 0, 1, pts)
                    filler()  # p0
                    for m in carry:
                        m()
                    carry = []
                    s_pair(g, hp, 0, 2, 3, pts)
                    filler()  # p1
                    s_pair(g, hp, 1, 0, 1, pts)
                    filler()  # p2
                    s_pair(g, hp, 1, 2, 3, pts)
                    filler()  # p3
                    # AV qc0 head h0
                    yp00 = pyp.tile([128, 512], f32, tag="pyp", name=f"yp{g}_{h0}_0")
                    av_chain(g, h0, 0, range(4), 4, yp00, pts)
                    mul00 = norm_start(g, h0, 0, yp00)
                    s_pair(g, hp, 1, 4, 5, pts)
                    filler()  # p4
                    # AV qc0 head h1
                    yp10 = pyp.tile([128, 512], f32, tag="pyp", name=f"yp{g}_{h1}_0")
                    av_chain(g, h1, 0, range(4), 4, yp10, pts)
                    mul10 = norm_start(g, h1, 0, yp10)
                    s_pair(g, hp, 1, 6, 7, pts)
                    filler()  # p5
                    mul00()  # its broadcast is done; frees yp00 for yp11
                    # AV qc1 head h0, filler half inside the accumulation chain
                    yp01 = pyp.tile([128, 512], f32, tag="pyp", name=f"yp{g}_{h0}_1")
                    av_chain(g, h0, 1, range(4), 8, yp01, pts)
                    filler()  # p6
                    av_chain(g, h0, 1, range(4, 8), 8, yp01, pts)
                    mul01 = norm_start(g, h0, 1, yp01)
                    mul10()
                    # AV qc1 head h1
                    yp11 = pyp.tile([128, 512], f32, tag="pyp", name=f"yp{g}_{h1}_1")
                    av_chain(g, h1, 1, range(4), 8, yp11, pts)
                    filler()  # p7
                    av_chain(g, h1, 1, range(4, 8), 8, yp11, pts)
                    mul11 = norm_start(g, h1, 1, yp11)
                    carry = [mul01, mul11]

            # ---- epilogue: outproj(set 3) + store ----
            for m in carry:
                m()
            wp_tiles[(3, 1)] = load_wp(3, 1)  # hides under the cc0 groups
            for cc in range(2):
                for tt in range(NT):
                    outproj_group(yt[3 % 2], wp_tiles[(3, cc)], 3, cc, tt, psp)
                    if cc == 1:  # osb[tt] final -> overlap store with compute
                        nc.sync.dma_start(
                            out=out[tt * 128 : (tt + 1) * 128, :], in_=osb[tt]
                        )

    nc.compile()
    return nc


def kernel(x, W_attn, W_proj, _trace=False):
    if "nc" not in _CACHE:
        _CACHE["nc"] = _build()
    nc = _CACHE["nc"]

    xT = np.ascontiguousarray(np.transpose(np.asarray(x, np.float32), (0, 2, 1))).astype(BF16)
    waT = np.ascontiguousarray(np.asarray(W_attn, np.float32).T).astype(BF16)
    wpT = np.ascontiguousarray(np.asarray(W_proj, np.float32).T).astype(BF16)
    ii = np.arange(128)
    # P^T[k, q] keep (multiply by 1) where q >= k
    tri = (ii[None, :] >= ii[:, None]).astype(np.float32).astype(BF16)

    in_maps = [
        {"xT": xT[b], "waT": waT, "wpT": wpT, "tri": tri}
        for b in range(B)
    ]
    res = run_bass_kernel_spmd(nc, in_maps, core_ids=list(range(B)), trace=_trace)
    LAST["exec_time_ns"] = res.exec_time_ns
    LAST["mean_exec_time_ns"] = res.mean_exec_time_ns
    LAST["results"] = res
    return np.stack([res.results[b]["out"] for b in range(B)]).astype(np.float32)
